# revision 1
# baseline (speedup 1.0000x reference)
"""GraphSAGE (2-layer, mean aggregation) on 8 Trainium2 NeuronCores.

Sharding: nodes partitioned by dst range across 8 cores (graph parallel).
Layer-1 edge messages x[src] are a compile-time permutation, so the host
stages them as a contiguous stream (direct DMA at full bandwidth). Layer-2
messages h[src] are batch-gathered from the AllGathered node-major h table
with one dma_gather per (group, table-half) — int16 gather indices only
reach 32767, so edges are chunked per (dst-tile, lo/hi table half), where
the hi half reads htab[7232:] with rebased indices. Segment-sum runs as
TensorE matmuls against one-hot dst-selection tiles generated on-device by
one DVE iota==dst compare per group. Dense SAGE transforms emit node-major
blocks directly (PE contracts channels with the node-block as the
stationary operand; bias via a rank-1 e0 x b matmul), so no transposes are
needed. Node rows use a (n%125)*40+n//125 permuted layout so every DMA
write is contiguous; the host un-permutes. All data paths are bf16 except
f32 PSUM accumulation and the f32 output. Weights are replicated.
"""

import ml_dtypes
import numpy as np

import concourse.bass as bass
import concourse.library_config as library_config
import concourse.mybir as mybir
import concourse.tile as tile
from concourse.bass_utils import run_bass_kernel_spmd
from concourse.library_overlay import lower_extended_insts
from concourse.tile import ScopedClock

# ---------------------------------------------------------------------------
# Workarounds for this container's walrus codegen: instructions can carry at
# most one sync-wait command ("Too many sync wait commands" otherwise), and
# Drain-based barriers reject waits entirely.
# ---------------------------------------------------------------------------


def _drain_and_barrier(self, tick_clock, wait_clock):
    nop_inst = self.nc.sync.nop(nofuse=True, hint="pre_drain_waits")
    wait_clock.add_sem_waits(
        nop_inst.ins, ScopedClock({None: tick_clock.global_clock})
    )
    si = nop_inst.ins.sync_info
    waits = list(si.on_wait) if si and si.on_wait else []
    if len(waits) > 1:
        si.on_wait = waits[:1]
        for w in waits[1:]:
            extra = self.nc.sync.nop(nofuse=True, hint="pre_drain_waits_x")
            extra.ins.sync_info = type(si)(on_wait=[w], on_update=[])
    self.nc.sync.drain()
    self.nc.all_engine_barrier(sem_only=True)
    assert self.sems is not None
    popped = self.nc._tile_sem_poison_stack.pop()
    assert popped is self._sem_poison
    self.nc.clear_and_free_semaphores(list(self.sems.allocated().values()))
    self.nc.all_engine_barrier(sem_only=True)


tile.TileContext._drain_and_barrier = _drain_and_barrier


def _split_multi_waits(nc, maxw=1):
    """Move excess sync-waits onto same-engine NOPs inserted before."""
    n = 0
    for blk in nc.m.functions[0].blocks:
        il = blk.instructions
        i = 0
        while i < len(il):
            inst = il[i]
            si = inst.sync_info
            waits = list(si.on_wait) if si and si.on_wait else []
            if len(waits) > maxw:
                si.on_wait = waits[-maxw:]
                for w in waits[:-maxw]:
                    nop = mybir.InstNoOp(
                        name=f"wsplit-{n}",
                        engine=inst.engine,
                        sync_info=mybir.SyncInfo(on_wait=[w], on_update=[]),
                    )
                    n += 1
                    il.insert(i, nop)
                    i += 1
            i += 1


# ---------------------------------------------------------------------------

N = 40000
E = 640000
C = 128          # in/hidden channels
O = 121          # out channels
NCORES = 8
NLOC = N // NCORES       # 5000 dst nodes per core
DT = 50                  # dst nodes per aggregation tile
NT = NLOC // DT          # 100 dst tiles per core
GT = 10                  # tiles per pipeline group
NG = NT // GT            # 10 groups per layer
GCOL = GT * DT           # 500 agg columns per group
DBLK = 125               # nodes per dense output block
NBLK = NLOC // DBLK      # 40 dense blocks
BPG = GCOL // DBLK       # 4 dense blocks per group
P = 128                  # edges per chunk (matmul contraction dim)
DT2 = 100                # layer-2 dst tile (coarser: less ceil padding)
NT2 = NLOC // DT2        # 50 layer-2 tiles
GT2 = GCOL // DT2        # 5 layer-2 tiles per group
I16MAX = 32768           # dma_gather int16 index limit
HIBASE = N - I16MAX      # hi table reads htab[HIBASE:]
F32 = mybir.dt.float32
BF16 = mybir.dt.bfloat16
I16 = mybir.dt.int16

_cache = {}
DEBUG = False
PHASE = 5  # 1=L1, 2=+AllGather, 3=+L2 gathers, 4=+L2 agg, 5=full


def _build(meta):
    """meta = (kc1, kp1, klo, khi, kp2) chunk structure, SPMD-identical."""
    key = (meta, DEBUG, PHASE)
    if key in _cache:
        return _cache[key]
    kc1, kp1, klo, khi, kp2 = meta   # klo/khi = A/B half chunk counts
    kc1 = np.array(kc1)
    klo = np.array(klo)
    khi = np.array(khi)
    kc2 = klo + khi
    coff1 = np.concatenate([[0], np.cumsum(kc1)])
    nch1 = int(coff1[-1])
    nlo = int(klo.sum())
    nhi = int(khi.sum())
    # per-group A/B chunk offsets for layer 2
    glo = np.concatenate([[0], np.cumsum(klo.reshape(NG, GT2).sum(axis=1))])
    ghi = np.concatenate([[0], np.cumsum(khi.reshape(NG, GT2).sum(axis=1))])

    nc = bass.Bass(num_swdge_queues=4)
    mstream = nc.dram_tensor("mstream", [P, nch1 * C], BF16, kind="ExternalInput")
    xT = nc.dram_tensor("xT", [C, NLOC], BF16, kind="ExternalInput")
    invc = nc.dram_tensor("invc", [P, NLOC], BF16, kind="ExternalInput")
    iota1 = nc.dram_tensor("iota1", [P, DT * kp1], BF16, kind="ExternalInput")
    iota2 = nc.dram_tensor("iota2", [P, DT2 * kp2], BF16, kind="ExternalInput")
    dval1 = nc.dram_tensor("dval1", [P, NT * kp1], BF16, kind="ExternalInput")
    dval2 = nc.dram_tensor("dval2", [P, NT2 * kp2], BF16, kind="ExternalInput")
    idxlo = nc.dram_tensor("idxlo", [P, nlo * 8], I16, kind="ExternalInput")
    idxhi = nc.dram_tensor("idxhi", [P, max(nhi, 1) * 8], I16, kind="ExternalInput")
    w1lT = nc.dram_tensor("w1lT", [C, C], BF16, kind="ExternalInput")
    w1rT = nc.dram_tensor("w1rT", [C, C], BF16, kind="ExternalInput")
    w2lT = nc.dram_tensor("w2lT", [C, O], BF16, kind="ExternalInput")
    w2rT = nc.dram_tensor("w2rT", [C, O], BF16, kind="ExternalInput")
    b1c = nc.dram_tensor("b1c", [C, 1], F32, kind="ExternalInput")
    b1row = nc.dram_tensor("b1row", [P, C], BF16, kind="ExternalInput")
    b2row = nc.dram_tensor("b2row", [P, O], BF16, kind="ExternalInput")
    out = nc.dram_tensor("out", [NLOC, O], F32, kind="ExternalOutput")
    if DEBUG:
        agg1dbg = nc.dram_tensor("agg1dbg", [C, NLOC], BF16, kind="ExternalOutput")
        agg2dbg = nc.dram_tensor("agg2dbg", [C, NLOC], BF16, kind="ExternalOutput")
        hdbg = nc.dram_tensor("hdbg", [NLOC, C], BF16, kind="ExternalOutput")

    with tile.TileContext(nc) as tc:
        with (
            tc.tile_pool(name="const", bufs=1) as cpool,
            tc.tile_pool(name="feat", bufs=1) as fpool,
            tc.tile_pool(name="msg", bufs=4) as mpool,
            tc.tile_pool(name="oh", bufs=4) as opool,
            tc.tile_pool(name="stg", bufs=2) as spool,
            tc.tile_pool(name="psum_a", bufs=2, space="PSUM") as pa,
            tc.tile_pool(name="psum_d", bufs=2, space="PSUM") as pd,
            tc.tile_pool(name="psum_n", bufs=3, space="PSUM") as pn,
            tc.tile_pool(name="dram", bufs=1, space="DRAM") as dpool,
        ):
            # ---- resident tiles -------------------------------------------
            xT_s = fpool.tile([C, NLOC], BF16)
            invc_s = fpool.tile([P, NLOC], BF16)
            iota1_s = fpool.tile([P, DT * kp1], BF16)
            iota2_s = fpool.tile([P, DT2 * kp2], BF16)
            dval1_s = fpool.tile([P, NT * kp1], BF16)
            dval2_s = fpool.tile([P, NT2 * kp2], BF16)
            idxlo_s = fpool.tile([P, nlo * 8], I16)
            idxhi_s = fpool.tile([P, max(nhi, 1) * 8], I16)
            aggT_s = fpool.tile([C, NLOC], BF16)
            hT_s = fpool.tile([C, NLOC], BF16)
            w1lT_s = cpool.tile([C, C], BF16)
            w1rT_s = cpool.tile([C, C], BF16)
            w2lT_s = cpool.tile([C, O], BF16)
            w2rT_s = cpool.tile([C, O], BF16)
            b1c_s = cpool.tile([C, 1], F32)
            b1row_s = cpool.tile([P, C], BF16)
            b2row_s = cpool.tile([P, O], BF16)
            e0_s = cpool.tile([P, DBLK], BF16)
            zer_s = cpool.tile([P, 1], F32)

            hloc = dpool.tile([NLOC, C], BF16)
            htab = dpool.tile([N, C], BF16, addr_space="Shared")

            nc.sync.dma_start(out=iota1_s[:], in_=iota1[:])
            nc.sync.dma_start(out=dval1_s[:], in_=dval1[:])
            nc.sync.dma_start(out=w1lT_s[:], in_=w1lT[:])
            nc.sync.dma_start(out=w1rT_s[:], in_=w1rT[:])
            nc.sync.dma_start(out=b1c_s[:], in_=b1c[:])
            nc.sync.dma_start(out=b1row_s[:], in_=b1row[:])
            nc.gpsimd.memset(e0_s[:], 0.0)
            nc.gpsimd.memset(e0_s[0:1, :], 1.0)
            nc.gpsimd.memset(zer_s[:], 0.0)
            nc.gpsimd.load_library(library_config.mlp)

            hloc_v = hloc[:, :].rearrange("(p q) c -> p q c", q=NBLK)
            out_v = out[:, :].rearrange("(p q) o -> p q o", q=NBLK)

            def onehot(g, iota_s, dval_s, kp, dt, gt):
                """One DVE is_equal builds the group's one-hot block:
                oh[p, t*dt*kp + d*kp + j] = (dst_off(tile t, chunk j, lane p) == d)."""
                oh = opool.tile([P, gt * dt * kp], BF16, tag="oh")
                oh4 = oh[:, :].rearrange(
                    "p (t d j) -> p t d j", t=gt, d=dt, j=kp
                )
                iota4 = (
                    iota_s[:, :]
                    .rearrange("p (d j) -> p d j", j=kp)
                    .unsqueeze(1)
                    .to_broadcast((P, gt, dt, kp))
                )
                dval4 = (
                    dval_s[:, gt * kp * g : gt * kp * (g + 1)]
                    .rearrange("p (t j) -> p t j", j=kp)
                    .unsqueeze(2)
                    .to_broadcast((P, gt, dt, kp))
                )
                nc.vector.tensor_tensor(
                    out=oh4, in0=iota4, in1=dval4, op=mybir.AluOpType.is_equal
                )
                return oh4

            def aggregate2(g, oh4, lhs, kcs, dt, gt):
                ps = pa.tile([C, GCOL], F32, space="PSUM")
                for t in range(gt):
                    tt = gt * g + t
                    k_t = int(kcs[tt])
                    for k in range(k_t):
                        nc.tensor.matmul(
                            out=ps[:, t * dt : (t + 1) * dt],
                            lhsT=lhs(tt, k),
                            rhs=oh4[:, t, :, k],
                            start=(k == 0),
                            stop=(k == k_t - 1),
                        )
                cols = slice(g * GCOL, (g + 1) * GCOL)
                nc.scalar.activation(
                    aggT_s[:, cols], ps[:], mybir.ActivationFunctionType.Copy
                )
                nc.vector.tensor_mul(
                    out=aggT_s[:, cols], in0=aggT_s[:, cols], in1=invc_s[:, cols]
                )

            def aggregate(g, oh4, big, kcs, column):
                """Accumulate the group's [C, 500] segment-sum and normalize."""
                ps = pa.tile([C, GCOL], F32, space="PSUM")
                for t in range(GT):
                    tt = GT * g + t
                    k_t = int(kcs[tt])
                    for k in range(k_t):
                        jc = column(tt, k)
                        nc.tensor.matmul(
                            out=ps[:, t * DT : (t + 1) * DT],
                            lhsT=big[:, jc * C : (jc + 1) * C],
                            rhs=oh4[:, t, :, k],
                            start=(k == 0),
                            stop=(k == k_t - 1),
                        )
                cols = slice(g * GCOL, (g + 1) * GCOL)
                nc.scalar.activation(
                    aggT_s[:, cols], ps[:], mybir.ActivationFunctionType.Copy
                )
                nc.vector.tensor_mul(
                    out=aggT_s[:, cols], in0=aggT_s[:, cols], in1=invc_s[:, cols]
                )
                return cols

            # ---- layer 1 ---------------------------------------------------
            for g in range(NG):
                j0, j1 = int(coff1[GT * g]), int(coff1[GT * (g + 1)])
                kg = j1 - j0
                big = mpool.tile([P, kg * C], BF16, tag="big")
                nc.sync.dma_start(
                    out=big[:], in_=mstream[:, j0 * C : j1 * C]
                )
                if g == 0:
                    # after group 0's slab so that DMA issues first, but
                    # before any consumer of xT/invc in program order
                    nc.sync.dma_start(out=xT_s[:], in_=xT[:])
                    nc.sync.dma_start(out=invc_s[:], in_=invc[:])
                oh4 = onehot(g, iota1_s, dval1_s, kp1, DT, GT)
                cols = aggregate(
                    g, oh4, big, kc1, lambda tt, k: int(coff1[tt]) - j0 + k
                )
                # channel-major dense -> hT
                ph = pd.tile([C, GCOL], F32, space="PSUM")
                nc.tensor.matmul(
                    out=ph[:], lhsT=w1lT_s[:], rhs=aggT_s[:, cols],
                    start=True, stop=False,
                )
                nc.tensor.matmul(
                    out=ph[:], lhsT=w1rT_s[:], rhs=xT_s[:, cols],
                    start=False, stop=True,
                )
                nc.scalar.activation(
                    hT_s[:, cols], ph[:],
                    mybir.ActivationFunctionType.Relu, bias=b1c_s[:, :1],
                )
                # node-major dense -> h blocks -> hloc (permuted rows)
                hstg = spool.tile([DBLK, BPG * C], BF16, tag="hstg")
                for b in range(BPG):
                    nb = slice((g * BPG + b) * DBLK, (g * BPG + b + 1) * DBLK)
                    pnb = pn.tile([DBLK, C], F32, space="PSUM")
                    nc.tensor.matmul(
                        out=pnb[:], lhsT=aggT_s[:, nb], rhs=w1lT_s[:],
                        start=True, stop=False,
                    )
                    nc.tensor.matmul(
                        out=pnb[:], lhsT=xT_s[:, nb], rhs=w1rT_s[:],
                        start=False, stop=False,
                    )
                    nc.tensor.matmul(
                        out=pnb[:], lhsT=e0_s[:], rhs=b1row_s[:],
                        start=False, stop=True,
                    )
                    nc.scalar.activation(
                        hstg[:, b * C : (b + 1) * C], pnb[:],
                        mybir.ActivationFunctionType.Relu,
                        bias=zer_s[:DBLK, :1],
                    )
                nc.sync.dma_start(
                    out=hloc_v[:, g * BPG : (g + 1) * BPG, :],
                    in_=hstg[:, :].rearrange("p (b c) -> p b c", c=C),
                )
                if DEBUG:
                    hdbg_v = hdbg[:, :].rearrange("(p q) c -> p q c", q=NBLK)
                    nc.sync.dma_start(
                        out=hdbg_v[:, g * BPG : (g + 1) * BPG, :],
                        in_=hstg[:, :].rearrange("p (b c) -> p b c", c=C),
                    )
            if DEBUG:
                nc.sync.dma_start(out=agg1dbg[:, :], in_=aggT_s[:])

            nc.sync.dma_start(out=iota2_s[:], in_=iota2[:])
            nc.sync.dma_start(out=dval2_s[:], in_=dval2[:])
            nc.sync.dma_start(out=idxlo_s[:], in_=idxlo[:])
            nc.sync.dma_start(out=idxhi_s[:], in_=idxhi[:])
            nc.sync.dma_start(out=w2lT_s[:], in_=w2lT[:])
            nc.sync.dma_start(out=w2rT_s[:], in_=w2rT[:])
            nc.sync.dma_start(out=b2row_s[:], in_=b2row[:])

            # ---- layer 2 ---------------------------------------------------
            loff_lo = np.concatenate([[0], np.cumsum(klo)])
            loff_hi = np.concatenate([[0], np.cumsum(khi)])
            _nreg = {}
            qrr = [0]

            def nreg(v):
                if v not in _nreg:
                    _nreg[v] = nc.gpsimd.to_reg(v)
                return _nreg[v]

            # SWDGE descriptor-ring capacity bounds one call at 1024 idxs;
            # split each half into <=GMAX-chunk calls on rotating queues.
            GMAX = 8

            def gather_half(big, kgh, base_col, idx_s, goff, tab):
                # tab is an AP slice of htab
                for c0 in range(0, kgh, GMAX):
                    ncall = min(GMAX, kgh - c0)
                    col = base_col + c0
                    nc.gpsimd.dma_gather(
                        out_ap=big[
                            :, col * C : (col + ncall) * C
                        ].rearrange("p (k c) -> p k c", c=C),
                        in_ap=tab,
                        idxs_ap=idx_s[
                            :, (goff + c0) * 8 : (goff + c0 + ncall) * 8
                        ],
                        num_idxs=ncall * P,
                        num_idxs_reg=nreg(ncall * P),
                        elem_size=C,
                        queue_num=qrr[0] % 4,
                    )
                    qrr[0] += 1

            def process(g):
                kglo = int(glo[g + 1] - glo[g])
                kghi = int(ghi[g + 1] - ghi[g])
                big = mpool.tile([P, (kglo + kghi) * C], BF16, tag="big")
                gather_half(big, kglo, 0, idxlo_s, int(glo[g]), htab[:I16MAX, :])
                gather_half(
                    big, kghi, kglo, idxhi_s, int(ghi[g]), htab[HIBASE:, :]
                )
                if PHASE < 4:
                    return

                def lhs2(tt, k, g=g, big=big, kglo=kglo):
                    if k < klo[tt]:
                        jc = int(loff_lo[tt] - glo[g]) + k
                    else:
                        jc = (
                            kglo
                            + int(loff_hi[tt] - ghi[g])
                            + (k - int(klo[tt]))
                        )
                    return big[:, jc * C : (jc + 1) * C]

                oh4 = onehot(g, iota2_s, dval2_s, kp2, DT2, GT2)
                aggregate2(g, oh4, lhs2, kc2, DT2, GT2)
                if PHASE < 5:
                    return
                # node-major dense -> out blocks (permuted rows)
                ostg = spool.tile([DBLK, BPG * O], F32, tag="ostg")
                for b in range(BPG):
                    nb = slice((g * BPG + b) * DBLK, (g * BPG + b + 1) * DBLK)
                    pnb = pn.tile([DBLK, O], F32, space="PSUM")
                    nc.tensor.matmul(
                        out=pnb[:], lhsT=aggT_s[:, nb], rhs=w2lT_s[:],
                        start=True, stop=False,
                    )
                    nc.tensor.matmul(
                        out=pnb[:], lhsT=hT_s[:, nb], rhs=w2rT_s[:],
                        start=False, stop=False,
                    )
                    nc.tensor.matmul(
                        out=pnb[:], lhsT=e0_s[:], rhs=b2row_s[:],
                        start=False, stop=True,
                    )
                    nc.scalar.activation(
                        ostg[:, b * O : (b + 1) * O], pnb[:],
                        mybir.ActivationFunctionType.Copy,
                    )
                nc.sync.dma_start(
                    out=out_v[:, g * BPG : (g + 1) * BPG, :],
                    in_=ostg[:, :].rearrange("p (b o) -> p b o", o=O),
                )

            # software pipeline: A-half gathers for group g+1 are issued
            # before the B-half of group g, so A transfers overlap the L1
            # tail / AG-B; AG-B is emitted after the first A batch so the
            # gpsimd queue is not blocked behind its hlocB wait.
            if PHASE >= 2:
                nc.gpsimd.collective_compute(
                    "AllGather",
                    mybir.AluOpType.bypass,
                    replica_groups=[list(range(NCORES))],
                    ins=[hloc.opt()],
                    outs=[htab.opt()],
                )
            if PHASE >= 3:
                for g in range(NG):
                    process(g)
            if DEBUG:
                nc.sync.dma_start(out=agg2dbg[:, :], in_=aggT_s[:])

    _split_multi_waits(nc)
    lower_extended_insts(nc)
    _cache[key] = nc
    return nc


def _wrap16(vals):
    """dma_gather index layout: linear idx i at partition i%16, col i//16,
    replicated across the 8 Q7-core partition stripes."""
    v = np.asarray(vals, np.int16).reshape(-1, 16)
    return np.tile(v.T, (P // 16, 1))


def _prepare(x, edge_index, W1l, b1l, W1r, b1r, W2l, b2l, W2r, b2r):
    src = np.asarray(edge_index[0], dtype=np.int64)
    dst = np.asarray(edge_index[1], dtype=np.int64)
    x = np.ascontiguousarray(np.asarray(x, dtype=np.float32))
    x_bf = x.astype(ml_dtypes.bfloat16)

    cnt = np.bincount(dst, minlength=N).astype(np.float32)
    inv_cnt = 1.0 / np.maximum(cnt, 1.0)

    order = np.argsort(dst, kind="stable")
    src_sorted = src[order].astype(np.int64)
    dst_sorted = dst[order]

    # permuted node-row layout: local node n lives at row rho(n)
    loc = np.arange(NLOC)
    rho_loc = (loc % DBLK) * NBLK + loc // DBLK
    gl = np.arange(N)
    rho_glob = ((gl // NLOC) * NLOC + rho_loc[gl % NLOC]).astype(np.int64)
    row2 = rho_glob[src_sorted]          # layer-2 htab row per sorted edge
    is_lo = row2 < I16MAX

    # global tile boundaries (dst-sorted)
    tile_edges = np.searchsorted(dst_sorted, np.arange(0, N + 1, DT))
    counts = np.diff(tile_edges).reshape(NCORES, NT)
    kc1 = np.maximum(np.ceil(counts.max(axis=0) / P).astype(int), 1)
    kp1 = int(kc1.max())
    coff1 = np.concatenate([[0], np.cumsum(kc1)])
    nch1 = int(kc1.sum())

    # layer-2 lo/hi chunk counts per DT2 tile (max over cores)
    tile_edges2 = np.searchsorted(dst_sorted, np.arange(0, N + 1, DT2))
    counts2 = np.diff(tile_edges2).reshape(NCORES, NT2)
    nlo_ct = np.zeros((NCORES, NT2), np.int64)
    for c in range(NCORES):
        for t in range(NT2):
            gidx = c * NT2 + t
            nlo_ct[c, t] = int(
                is_lo[tile_edges2[gidx] : tile_edges2[gidx + 1]].sum()
            )
    nhi_ct = counts2 - nlo_ct
    klo = np.maximum(np.ceil(nlo_ct.max(axis=0) / P).astype(int), 1)
    khi = np.maximum(np.ceil(nhi_ct.max(axis=0) / P).astype(int), 1)
    kc2 = klo + khi
    kp2 = int(kc2.max())
    nlo = int(klo.sum())
    nhi = int(khi.sum())
    loff_lo = np.concatenate([[0], np.cumsum(klo)])
    loff_hi = np.concatenate([[0], np.cumsum(khi)])
    glo = np.concatenate([[0], np.cumsum(klo.reshape(NG, GT2).sum(axis=1))])
    ghi = np.concatenate([[0], np.cumsum(khi.reshape(NG, GT2).sum(axis=1))])

    meta = (
        tuple(int(v) for v in kc1), kp1,
        tuple(int(v) for v in klo), tuple(int(v) for v in khi), kp2,
    )

    def iota_arr(kp, dt):
        a = np.zeros((P, dt * kp), ml_dtypes.bfloat16)
        a[:, :] = np.repeat(np.arange(dt, dtype=np.float32), kp)[None, :]
        return a

    w1lT_np = np.ascontiguousarray(np.asarray(W1l, np.float32).T.astype(ml_dtypes.bfloat16))
    w1rT_np = np.ascontiguousarray(np.asarray(W1r, np.float32).T.astype(ml_dtypes.bfloat16))
    w2lT_np = np.ascontiguousarray(np.asarray(W2l, np.float32).T.astype(ml_dtypes.bfloat16))
    w2rT_np = np.ascontiguousarray(np.asarray(W2r, np.float32).T.astype(ml_dtypes.bfloat16))
    b1_np = np.asarray(b1l, np.float32) + np.asarray(b1r, np.float32)
    b2_np = np.asarray(b2l, np.float32) + np.asarray(b2r, np.float32)
    b1c_np = np.ascontiguousarray(b1_np[:, None])
    b1row_np = np.zeros((P, C), ml_dtypes.bfloat16)
    b1row_np[0, :] = b1_np.astype(ml_dtypes.bfloat16)
    b2row_np = np.zeros((P, O), ml_dtypes.bfloat16)
    b2row_np[0, :] = b2_np.astype(ml_dtypes.bfloat16)
    xT_full = np.ascontiguousarray(x.T.astype(ml_dtypes.bfloat16))
    iota1_np = iota_arr(kp1, DT)
    iota2_np = iota_arr(kp2, DT2)

    in_maps = []
    for c in range(NCORES):
        base = c * NLOC
        src1_cols = np.zeros((nch1, P), np.int64)    # layer-1 chunk src ids
        dval1_np = np.full((NT * kp1, P), -1.0, np.float32)
        dval2_np = np.full((NT2 * kp2, P), -1.0, np.float32)
        lo_rows = np.zeros((nlo, P), np.int16)
        hi_rows = np.zeros((max(nhi, 1), P), np.int16)
        for t in range(NT):
            gidx = c * NT + t
            e0, e1 = tile_edges[gidx], tile_edges[gidx + 1]
            s = src_sorted[e0:e1]
            d = (dst_sorted[e0:e1] - (base + t * DT)).astype(np.float32)
            o = np.argsort(s, kind="stable")
            s, d = s[o], d[o]

            # layer 1: all edges, packed to kc1[t] chunks
            k_t = int(kc1[t])
            fs = np.zeros(k_t * P, np.int64)
            fd = np.full(k_t * P, -1.0, np.float32)
            fs[: len(s)] = s
            fd[: len(s)] = d
            src1_cols[coff1[t] : coff1[t + 1]] = fs.reshape(k_t, P)
            dval1_np[t * kp1 : t * kp1 + k_t] = fd.reshape(k_t, P)

        for t in range(NT2):
            gidx = c * NT2 + t
            e0, e1 = tile_edges2[gidx], tile_edges2[gidx + 1]
            s = src_sorted[e0:e1]
            r2 = row2[e0:e1]
            a_m = is_lo[e0:e1]
            d = (dst_sorted[e0:e1] - (base + t * DT2)).astype(np.float32)
            o = np.argsort(s, kind="stable")
            d, r2, lo_m = d[o], r2[o], a_m[o]

            # layer 2: lo chunks then hi chunks
            k_l, k_h = int(klo[t]), int(khi[t])
            fr = np.zeros(k_l * P, np.int16)
            fd2 = np.full((k_l + k_h) * P, -1.0, np.float32)
            n_l = int(lo_m.sum())
            fr[:n_l] = r2[lo_m].astype(np.int16)
            fd2[:n_l] = d[lo_m]
            lo_rows[loff_lo[t] : loff_lo[t + 1]] = fr.reshape(k_l, P)
            if k_h > 0:
                frh = np.zeros(k_h * P, np.int16)
                n_h = int((~lo_m).sum())
                frh[:n_h] = (r2[~lo_m] - HIBASE).astype(np.int16)
                fd2[k_l * P : k_l * P + n_h] = d[~lo_m]
                hi_rows[loff_hi[t] : loff_hi[t + 1]] = frh.reshape(k_h, P)
            dval2_np[t * kp2 : t * kp2 + k_l + k_h] = fd2.reshape(k_l + k_h, P)

        # layer-1 message stream: x[src], chunk-major
        mstream_np = (
            x_bf[src1_cols].transpose(1, 0, 2).reshape(P, nch1 * C)
        )
        # dma_gather index arrays, wrapped per group
        idxlo_np = np.zeros((P, nlo * 8), np.int16)
        idxhi_np = np.zeros((P, max(nhi, 1) * 8), np.int16)
        for g in range(NG):
            blk = lo_rows[glo[g] : glo[g + 1]].reshape(-1)
            idxlo_np[:, glo[g] * 8 : glo[g + 1] * 8] = _wrap16(blk)
            if ghi[g + 1] > ghi[g]:
                blk = hi_rows[ghi[g] : ghi[g + 1]].reshape(-1)
                idxhi_np[:, ghi[g] * 8 : ghi[g + 1] * 8] = _wrap16(blk)

        in_maps.append(
            {
                "mstream": np.ascontiguousarray(mstream_np),
                "xT": np.ascontiguousarray(xT_full[:, base : base + NLOC]),
                "invc": np.broadcast_to(
                    inv_cnt[base : base + NLOC].astype(ml_dtypes.bfloat16),
                    (P, NLOC),
                ).copy(),
                "iota1": iota1_np,
                "iota2": iota2_np,
                "dval1": np.ascontiguousarray(
                    dval1_np.T.astype(ml_dtypes.bfloat16)
                ),
                "dval2": np.ascontiguousarray(
                    dval2_np.T.astype(ml_dtypes.bfloat16)
                ),
                "idxlo": idxlo_np,
                "idxhi": idxhi_np,
                "w1lT": w1lT_np,
                "w1rT": w1rT_np,
                "w2lT": w2lT_np,
                "w2rT": w2rT_np,
                "b1c": b1c_np,
                "b1row": b1row_np,
                "b2row": b2row_np,
            }
        )
    return meta, rho_loc, in_maps


def _install_profile_hook():
    """The stripped agent image lacks antenv.axon_hooks; synthesize it and
    register the ctypes NTFF profile hook so trace=True works."""
    import sys
    import types

    if "antenv.axon_hooks" in sys.modules:
        return
    import antenv

    mod = types.ModuleType("antenv.axon_hooks")
    state = {"hook": None}
    mod.set_axon_ntff_profile_hook = lambda h: state.update(hook=h)
    mod.get_axon_ntff_profile_hook = lambda: state["hook"]
    sys.modules["antenv.axon_hooks"] = mod
    antenv.axon_hooks = mod

    from trn_agent_boot.trn_boot import _ntff_profile_via_ctypes

    mod.set_axon_ntff_profile_hook(
        _ntff_profile_via_ctypes("/opt/axon/libaxon_pjrt.so")
    )

    import concourse.bass_utils as bu

    bu.upload_artifacts = lambda tmpdir: tmpdir  # no remote bucket here


def kernel(trace=False, **inputs):
    if trace:
        _install_profile_hook()
    meta, rho_loc, in_maps = _prepare(**inputs)
    nc = _build(meta)
    res = run_bass_kernel_spmd(nc, in_maps, list(range(NCORES)), trace=trace)
    out = np.concatenate(
        [res.results[c]["out"][rho_loc] for c in range(NCORES)], axis=0
    )
    if trace:
        return out, res
    return out

